# revision 40
# baseline (speedup 1.0000x reference)
"""RNN-T JointNet kernel for 8 Trainium2 NeuronCores.

Math: out[b,t,u,:] = gelu_tanh(concat(enc[b,t], dec[b,u])) @ W_fc^T + b_fc
Since gelu is elementwise, gelu(concat(a,b)) = concat(gelu(a), gelu(b)), so
  out[b,t,u,:] = P_enc[b,t,:] + P_dec[b,u,:]
with P_enc = gelu(enc) @ W_fc[:, :512]^T + b_fc  (small matmul, (B,T,V))
     P_dec = gelu(dec) @ W_fc[:, 512:]^T         (small matmul, (B,U,V))
The dominant cost is streaming the (B,T,U,V) = 310MB f32 output to HBM
(~111us/core at 358 GB/s); everything else must hide under the stores.

Sharding: 8 cores = 4 batches x 2 u-halves. Core c -> b = c//2, u-range
[(c%2)*52, (c%2)*52+52) of U padded 101->104. Full T=300 per core.

Inputs/weights are pre-cast to bf16 and pre-tiled on the host so every
input lands in one contiguous-per-partition DMA; the weight DMA is
dispatched first since it gates the matmul phase. A burst of dummy
transposes keeps the PE HAM clock-gate open (2.4GHz) before the real
matmuls. Each P_enc/P_dec matmul group accumulates both v-chunks into one
[128,640] PSUM tile (shared tag with the loop broadcasts; 3 bufs + 2
transpose banks = 8 PSUM banks). P_dec rows are relayouted to a
partition-{0,32,64} row tile with one SBUF->SBUF DMA (no DRAM bounce).
Per u the PE broadcasts one P_dec row into PSUM (K=1 bf16 matmuls); one
DVE op adds pe2 (both 128-row t-chunks, stride-0 broadcast of the PSUM
tile) into the otA tile, ACT copies the 44-row tail slice to SBUF, and
gpsimd adds the t-tail for the whole 4-u block. Stores: one 2.62MB (4D AP
over both t-chunks) + one 0.45MB DMA per block, alternating HWDGE rings.
"""

import numpy as np

B, T, U = 4, 300, 101
D = 512
V = 640
UCORE = 52  # u rows per core (U padded to 104)
NCORES = 8
UB = 4  # u rows per store block (52 = 13 * 4)
RG = 18  # u rows per row-group partition (groups at partitions 0/32/64)
NWARM = 40  # dummy PE transposes to hold the HAM clock-gate open

LAST_RESULT = None  # BassKernelResults of the most recent run (for test.py)
RUN_KWARGS = {}  # extra kwargs test.py may inject (e.g. tmpdir for traces)

_cache = {}


def _build():
    import concourse.mybir as mybir
    from concourse import bacc, masks
    from concourse.tile import TileContext

    f32 = mybir.dt.float32
    bf16 = mybir.dt.bfloat16
    AF = mybir.ActivationFunctionType

    nc = bacc.Bacc()
    # host pre-tiled: enc[p, c, :] = gelu-input row t = c*128+p (zero-padded)
    enc_d = nc.dram_tensor("enc", [128, 3, D], bf16, kind="ExternalInput")
    dec_d = nc.dram_tensor("dec", [UCORE, D], bf16, kind="ExternalInput")
    # host pre-tiled: wT[p, c, :] = W_fc.T row d = c*128+p
    wT_d = nc.dram_tensor("wT", [128, 8, V], bf16, kind="ExternalInput")
    bias_d = nc.dram_tensor("bias", [1, V], bf16, kind="ExternalInput")
    # outputs laid out exactly like the SBUF tiles so every store is one
    # fully contiguous DRAM write (best HBM locality); host un-permutes.
    # outA[bi, p, c, j, v] = out[t = c*128 + p, u = 4*bi + j, v]
    # outB[bi, p, j, v]    = out[t = 256 + p,   u = 4*bi + j, v]
    NBLK = UCORE // UB
    outA_d = nc.dram_tensor("outA", [NBLK, 128, 2, UB, V], f32, kind="ExternalOutput")
    outB_d = nc.dram_tensor("outB", [NBLK, 44, UB, V], f32, kind="ExternalOutput")

    tchunks = [(0, 128), (128, 128), (256, 44)]
    vchunks = [(0, 512), (512, V - 512)]

    with TileContext(nc) as tc:
        with (
            tc.tile_pool(name="const", bufs=1) as constp,
            tc.tile_pool(name="work", bufs=2) as work,
            tc.tile_pool(name="persist", bufs=1) as persist,
            tc.tile_pool(name="outpA", bufs=3) as outpA,
            tc.tile_pool(name="outpB", bufs=3) as outpB,
            tc.tile_pool(name="bctp", bufs=3) as bctp,
            tc.tile_pool(name="psum", bufs=1, space="PSUM") as psum,
        ):
            # input loads: small gelu inputs first (they head the compute
            # chains), split across both HWDGE rings; w queued right behind
            dt_in = work.tile([128, D], bf16, tag="ld", name="dt_in")
            nc.sync.dma_start(dt_in[:UCORE, :], dec_d[:, :])
            et = work.tile([128, 3, D], bf16, tag="lde", name="et")
            nc.scalar.dma_start(et[:, :, :], enc_d[:, :, :])
            # w on the SWDGE (gpsimd) ring: dispatches early (no ACT table
            # loads ahead of it) and overlaps the HWDGE input loads
            w_bf = persist.tile([128, 8, V], bf16, tag="w")
            nc.gpsimd.dma_start(w_bf[:, :, :], wT_d[:, :, :])
            bias_sb = constp.tile([1, V], bf16)
            nc.scalar.dma_start(bias_sb[:], bias_d[:])

            ident = constp.tile([128, 128], bf16)
            masks.make_identity(nc, ident[:])
            # ones at base partitions 0/32/64 (matmul lhsT/rhs must share base)
            ones3 = constp.tile([65, 128], bf16)
            nc.gpsimd.memset(ones3[:], 1.0)

            # dummy PE ops: absorb the gpsimd-sem wait AND keep the PE HAM
            # activity window busy until real matmuls arrive, so they run at
            # 2.4GHz instead of the cold 1.2GHz
            warm = psum.tile([128, 128], bf16, tag="tr", bufs=2)
            for _ in range(NWARM):
                nc.tensor.transpose(warm[:, :], ident[:, :], ident[:, :])

            # gelu: dec first (heads the deeper P_dec->rows chain)
            gdec = persist.tile([128, D], bf16, tag="gdec")
            nc.scalar.activation(gdec[:UCORE, :], dt_in[:UCORE, :], AF.Gelu_apprx_tanh)
            genc = persist.tile([128, 3, D], bf16, tag="genc")
            nc.scalar.activation(genc[:, :, :], et[:, :, :], AF.Gelu_apprx_tanh)

            # transpose to [d, u] / [d, t]; psum->SBUF copies on the idle DVE
            gdecT = [persist.tile([128, UCORE], bf16, tag=f"gdecT{d}", name=f"gdecT{d}") for d in range(4)]
            gencT = [persist.tile([128, 384], bf16, tag=f"gencT{d}", name=f"gencT{d}") for d in range(4)]
            for dch in range(4):
                dsl = slice(dch * 128, (dch + 1) * 128)
                ps = psum.tile([128, 128], bf16, tag="tr", bufs=2)
                nc.tensor.transpose(ps[:, :UCORE], gdec[:UCORE, dsl], ident[:UCORE, :UCORE])
                nc.vector.tensor_copy(gdecT[dch][:, :UCORE], ps[:, :UCORE])
            for dch in range(4):
                dsl = slice(dch * 128, (dch + 1) * 128)
                for i in range(3):
                    ps = psum.tile([128, 128], bf16, tag="tr", bufs=2)
                    nc.tensor.transpose(ps[:, :], genc[:, i, dsl], ident[:, :])
                    nc.vector.tensor_copy(gencT[dch][:, i * 128 : (i + 1) * 128], ps[:, :])

            # P_dec [52,640] bf16 -> SBUF->SBUF DMA relayout to row tile at
            # partitions 0/32/64
            pd_bf = persist.tile([3 * RG, V], bf16, tag="pd")
            nc.gpsimd.memset(pd_bf[:, :], 0.0)  # rows 52-53 stay zero (pad)
            ps = psum.tile([128, V], f32, tag="bc", bufs=3)
            for v0, vn in vchunks:
                for d in range(4):
                    nc.tensor.matmul(
                        ps[:UCORE, v0 : v0 + vn],
                        gdecT[d][:, :UCORE],
                        w_bf[:, 4 + d, v0 : v0 + vn],
                        start=(d == 0),
                        stop=(d == 3),
                    )
            nc.vector.tensor_copy(pd_bf[:UCORE, :], ps[:UCORE, :])
            # NOTE: this dispatch waits on the pd copy INSIDE the sync FIFO —
            # keep it (and every dep-carrying dispatch) off the ACT engine,
            # whose FIFO must stay free for the pe2 copies
            rows = persist.tile([65, RG * V], bf16, tag="rows")
            nc.sync.dma_start(rows[0:65:32, :], pd_bf[:, :])

            # P_enc (with bias): t-chunks 0,1 packed in pe2. The 44-row t-tail
            # (t 256-299) is computed TWICE: once landing on partitions 0-43
            # (pe3a) and once on partitions 64-107 (pe3b, lhsT cols 192-300
            # with rows 192-255 duplicated). The tail store alternates between
            # them per block parity so its bytes split between the even
            # (p0-63) and odd (p64-127) SDMA engine groups — otherwise the
            # even engines carry all tail descriptors and bound the loop.
            pe2 = persist.tile([128, 2, V], f32, tag="pe2")
            pe3a = persist.tile([44, V], f32, tag="pe3a")
            pe3b = persist.tile([108, V], f32, tag="pe3b")

            def penc_group(t0, tn, copy):
                ps = psum.tile([128, V], f32, tag="bc", bufs=3, name="ps")
                for v0, vn in vchunks:
                    for d in range(4):
                        nc.tensor.matmul(
                            ps[:tn, v0 : v0 + vn],
                            gencT[d][:, t0 : t0 + tn],
                            w_bf[:, d, v0 : v0 + vn],
                            start=(d == 0),
                            stop=False,
                        )
                    nc.tensor.matmul(
                        ps[:tn, v0 : v0 + vn],
                        ones3[0:1, :tn],
                        bias_sb[:1, v0 : v0 + vn],
                        start=False,
                        stop=True,
                    )
                copy(ps)

            penc_group(0, 128, lambda ps: nc.scalar.copy(pe2[:, 0, :], ps[:, :]))
            penc_group(128, 128, lambda ps: nc.scalar.copy(pe2[:, 1, :], ps[:, :]))
            # the t-tail group (t 192-299, M=108) is emitted inside block 0 of
            # the loop so the first broadcast matmuls aren't queued behind it

            # main loop: 13 blocks of 4 u
            for bi, u0 in enumerate(range(0, UCORE, UB)):
                par = bi % 2
                if par == 0:
                    bct4 = bctp.tile([44, UB, V], f32, tag="bcta", name="bct4a", bufs=2)
                    otB = outpB.tile([44, UB, V], f32, tag="otBa", name="otBa", bufs=2)
                    psl, pe3s = slice(0, 44), pe3a[:, :]
                else:
                    bct4 = bctp.tile([108, UB, V], f32, tag="bctb", name="bct4b", bufs=2)
                    otB = outpB.tile([108, UB, V], f32, tag="otBb", name="otBb", bufs=2)
                    psl, pe3s = slice(64, 108), pe3b[64:108, :]
                otA = outpA.tile([128, 2, UB, V], f32, tag="otA", name="otA")
                for j in range(UB):
                    u = u0 + j
                    g, off = u // RG, (u % RG) * V
                    ps = psum.tile([128, V], f32, tag="bc", bufs=3)
                    for c0, cn in ((0, 512), (512, V - 512)):
                        nc.tensor.matmul(
                            ps[:, c0 : c0 + cn],
                            ones3[32 * g : 32 * g + 1, :128],
                            rows[32 * g : 32 * g + 1, off + c0 : off + c0 + cn],
                            start=True,
                            stop=True,
                        )
                    nc.scalar.copy(bct4[psl, j, :], ps[psl, :])
                    nc.vector.tensor_add(
                        otA[:, :, j, :],
                        pe2[:, :, :],
                        ps[:, :].unsqueeze(1).broadcast_to([128, 2, V]),
                    )
                if bi == 0:
                    # tail P_enc group: after block 0's broadcasts on the PE
                    penc_group(192, 108, lambda ps: nc.scalar.copy(pe3b[64:108, :], ps[64:108, :]))
                    # base-0 tail copy for even blocks via SBUF->SBUF DMA on
                    # the idle SWDGE ring (engines can't cross partitions)
                    nc.gpsimd.dma_start(pe3a[:, :], pe3b[64:108, :])
                # two pair-adds so otB is ready right after the j=3 copy
                # instead of one whole-block add later
                for j0 in (0, 2):
                    nc.gpsimd.tensor_add(
                        otB[psl, j0 : j0 + 2, :],
                        pe3s.unsqueeze(1).broadcast_to([44, 2, V]),
                        bct4[psl, j0 : j0 + 2, :],
                    )
                engA, engB = (nc.sync, nc.scalar) if bi % 2 == 0 else (nc.scalar, nc.sync)
                if bi == 0:
                    # first block: store per u-pair across both rings so the
                    # first store issues right after the second DVE add
                    nc.sync.dma_start(outA_d[bi, :, :, 0:2, :], otA[:, :, 0:2, :])
                    nc.scalar.dma_start(outA_d[bi, :, :, 2:4, :], otA[:, :, 2:4, :])
                    nc.sync.dma_start(outB_d[bi, :, :, :], otB[psl, :, :])
                elif bi < UCORE // UB - 1:
                    engA.dma_start(outA_d[bi, :, :, :, :], otA[:, :, :, :])
                    engB.dma_start(outB_d[bi, :, :, :], otB[psl, :, :])
                else:
                    # last block: split the big store across both rings so the
                    # drain is half as long
                    nc.sync.dma_start(outA_d[bi, :, 0, :, :], otA[:, 0, :, :])
                    nc.scalar.dma_start(outA_d[bi, :, 1, :, :], otA[:, 1, :, :])
                    nc.sync.dma_start(outB_d[bi, :, :, :], otB[psl, :, :])

    nc.compile()
    return nc


def kernel(encoder_outputs, decoder_outputs, W_fc, b_fc):
    global LAST_RESULT
    import os

    import ml_dtypes
    from concourse.bass_utils import run_bass_kernel_spmd

    bf = ml_dtypes.bfloat16
    enc = np.asarray(encoder_outputs, dtype=np.float32)
    dec = np.asarray(decoder_outputs, dtype=np.float32)

    # enc per batch -> [128, 3, 512] with row t = c*128 + p, zero padded
    enc_pad = np.zeros((B, 384, D), dtype=np.float32)
    enc_pad[:, :T, :] = enc
    enc_tiled = np.ascontiguousarray(
        enc_pad.reshape(B, 3, 128, D).transpose(0, 2, 1, 3)
    ).astype(bf)

    # W_fc.T -> [128, 8, 640] with row d = c*128 + p
    wT = np.asarray(W_fc, dtype=np.float32).T  # (1024, 640)
    wT_tiled = np.ascontiguousarray(
        wT.reshape(8, 128, V).transpose(1, 0, 2)
    ).astype(bf)

    bias = np.asarray(b_fc, dtype=np.float32)[None, :].astype(bf)

    dec_pad = np.zeros((B, 2 * UCORE, D), dtype=np.float32)
    dec_pad[:, :U, :] = dec
    dec_pad = dec_pad.astype(bf)

    if "nc" not in _cache:
        _cache["nc"] = _build()
    nc = _cache["nc"]

    in_maps = []
    for c in range(NCORES):
        b, uh = c // 2, c % 2
        in_maps.append(
            {
                "enc": enc_tiled[b],
                "dec": np.ascontiguousarray(dec_pad[b, uh * UCORE : (uh + 1) * UCORE]),
                "wT": wT_tiled,
                "bias": bias,
            }
        )

    res = run_bass_kernel_spmd(
        nc,
        in_maps,
        list(range(NCORES)),
        trace=bool(int(os.environ.get("KJ_TRACE", "0"))),
        **RUN_KWARGS,
    )
    LAST_RESULT = res

    out = np.empty((B, T, U, V), dtype=np.float32)
    for c in range(NCORES):
        b, uh = c // 2, c % 2
        # outA (13,128,2,4,640): [bi,p,cc,j,v] -> t = cc*128+p, u = 4*bi+j
        # outB (13,44,4,640):    [bi,p,j,v]    -> t = 256+p,    u = 4*bi+j
        outA = res.results[c]["outA"]
        outB = res.results[c]["outB"]
        cut = np.empty((T, UCORE, V), dtype=np.float32)
        cut[:256] = outA.transpose(2, 1, 0, 3, 4).reshape(256, UCORE, V)
        cut[256:] = outB.transpose(1, 0, 2, 3).reshape(44, UCORE, V)
        if uh == 0:
            out[b, :, :UCORE] = cut
        else:
            out[b, :, UCORE:U] = cut[:, : U - UCORE]
    return out


# revision 41
# speedup vs baseline: 1.0094x; 1.0094x over previous
"""RNN-T JointNet kernel for 8 Trainium2 NeuronCores.

Math: out[b,t,u,:] = gelu_tanh(concat(enc[b,t], dec[b,u])) @ W_fc^T + b_fc
Since gelu is elementwise, gelu(concat(a,b)) = concat(gelu(a), gelu(b)), so
  out[b,t,u,:] = P_enc[b,t,:] + P_dec[b,u,:]
with P_enc = gelu(enc) @ W_fc[:, :512]^T + b_fc  (small matmul, (B,T,V))
     P_dec = gelu(dec) @ W_fc[:, 512:]^T         (small matmul, (B,U,V))
The dominant cost is streaming the (B,T,U,V) = 310MB f32 output to HBM
(~111us/core at 358 GB/s); everything else must hide under the stores.

Sharding: 8 cores = 4 batches x 2 u-halves. Core c -> b = c//2, u-range
[(c%2)*52, (c%2)*52+52) of U padded 101->104. Full T=300 per core.

Inputs/weights are pre-cast to bf16 and pre-tiled on the host so every
input lands in one contiguous-per-partition DMA; the weight DMA is
dispatched first since it gates the matmul phase. A burst of dummy
transposes keeps the PE HAM clock-gate open (2.4GHz) before the real
matmuls. Each P_enc/P_dec matmul group accumulates both v-chunks into one
[128,640] PSUM tile (shared tag with the loop broadcasts; 3 bufs + 2
transpose banks = 8 PSUM banks). P_dec rows are relayouted to a
partition-{0,32,64} row tile with one SBUF->SBUF DMA (no DRAM bounce).
Per u the PE broadcasts one P_dec row into PSUM (K=1 bf16 matmuls); one
DVE op adds pe2 (both 128-row t-chunks, stride-0 broadcast of the PSUM
tile) into the otA tile, ACT copies the 44-row tail slice to SBUF, and
gpsimd adds the t-tail for the whole 4-u block. Stores: one 2.62MB (4D AP
over both t-chunks) + one 0.45MB DMA per block, alternating HWDGE rings.
"""

import numpy as np

B, T, U = 4, 300, 101
D = 512
V = 640
UCORE = 52  # u rows per core (U padded to 104)
NCORES = 8
UB = 4  # u rows per store block (52 = 13 * 4)
RG = 18  # u rows per row-group partition (groups at partitions 0/32/64)
NWARM = 45  # dummy PE transposes to hold the HAM clock-gate open

LAST_RESULT = None  # BassKernelResults of the most recent run (for test.py)
RUN_KWARGS = {}  # extra kwargs test.py may inject (e.g. tmpdir for traces)

_cache = {}


def _build():
    import concourse.mybir as mybir
    from concourse import bacc, masks
    from concourse.tile import TileContext

    f32 = mybir.dt.float32
    bf16 = mybir.dt.bfloat16
    AF = mybir.ActivationFunctionType

    nc = bacc.Bacc()
    # host pre-tiled: enc[p, c, :] = gelu-input row t = c*128+p (zero-padded)
    enc_d = nc.dram_tensor("enc", [128, 3, D], bf16, kind="ExternalInput")
    dec_d = nc.dram_tensor("dec", [UCORE, D], bf16, kind="ExternalInput")
    # host pre-tiled: wT[p, c, :] = W_fc.T row d = c*128+p
    wT_d = nc.dram_tensor("wT", [128, 8, V], bf16, kind="ExternalInput")
    bias_d = nc.dram_tensor("bias", [1, V], bf16, kind="ExternalInput")
    # outputs laid out exactly like the SBUF tiles so every store is one
    # fully contiguous DRAM write (best HBM locality); host un-permutes.
    # outA[bi, p, c, j, v] = out[t = c*128 + p, u = 4*bi + j, v]
    # outB[bi, p, j, v]    = out[t = 256 + p,   u = 4*bi + j, v]
    NBLK = UCORE // UB
    outA_d = nc.dram_tensor("outA", [NBLK, 128, 2, UB, V], f32, kind="ExternalOutput")
    outB_d = nc.dram_tensor("outB", [NBLK, 44, UB, V], f32, kind="ExternalOutput")

    tchunks = [(0, 128), (128, 128), (256, 44)]
    vchunks = [(0, 512), (512, V - 512)]

    with TileContext(nc) as tc:
        with (
            tc.tile_pool(name="const", bufs=1) as constp,
            tc.tile_pool(name="work", bufs=2) as work,
            tc.tile_pool(name="persist", bufs=1) as persist,
            tc.tile_pool(name="outpA", bufs=3) as outpA,
            tc.tile_pool(name="outpB", bufs=3) as outpB,
            tc.tile_pool(name="bctp", bufs=3) as bctp,
            tc.tile_pool(name="psum", bufs=1, space="PSUM") as psum,
        ):
            # input loads: small gelu inputs first (they head the compute
            # chains), split across both HWDGE rings; w queued right behind
            dt_in = work.tile([128, D], bf16, tag="ld", name="dt_in")
            nc.sync.dma_start(dt_in[:UCORE, :], dec_d[:, :])
            et = work.tile([128, 3, D], bf16, tag="lde", name="et")
            nc.scalar.dma_start(et[:, :, :], enc_d[:, :, :])
            # w on the SWDGE (gpsimd) ring: dispatches early (no ACT table
            # loads ahead of it) and overlaps the HWDGE input loads
            w_bf = persist.tile([128, 8, V], bf16, tag="w")
            nc.gpsimd.dma_start(w_bf[:, :, :], wT_d[:, :, :])
            bias_sb = constp.tile([1, V], bf16)
            nc.scalar.dma_start(bias_sb[:], bias_d[:])

            ident = constp.tile([128, 128], bf16)
            masks.make_identity(nc, ident[:])
            # ones at base partitions 0/32/64 (matmul lhsT/rhs must share base)
            ones3 = constp.tile([65, 128], bf16)
            nc.gpsimd.memset(ones3[:], 1.0)

            # dummy PE ops: absorb the gpsimd-sem wait AND keep the PE HAM
            # activity window busy until real matmuls arrive, so they run at
            # 2.4GHz instead of the cold 1.2GHz
            warm = psum.tile([128, 128], bf16, tag="tr", bufs=2)
            for _ in range(NWARM):
                nc.tensor.transpose(warm[:, :], ident[:, :], ident[:, :])

            # gelu: dec first (heads the deeper P_dec->rows chain)
            gdec = persist.tile([128, D], bf16, tag="gdec")
            nc.scalar.activation(gdec[:UCORE, :], dt_in[:UCORE, :], AF.Gelu_apprx_tanh)
            genc = persist.tile([128, 3, D], bf16, tag="genc")
            nc.scalar.activation(genc[:, :, :], et[:, :, :], AF.Gelu_apprx_tanh)

            # transpose to [d, u] / [d, t]; psum->SBUF copies on the idle DVE
            gdecT = [persist.tile([128, UCORE], bf16, tag=f"gdecT{d}", name=f"gdecT{d}") for d in range(4)]
            gencT = [persist.tile([128, 384], bf16, tag=f"gencT{d}", name=f"gencT{d}") for d in range(4)]
            for dch in range(4):
                dsl = slice(dch * 128, (dch + 1) * 128)
                ps = psum.tile([128, 128], bf16, tag="tr", bufs=2)
                nc.tensor.transpose(ps[:, :UCORE], gdec[:UCORE, dsl], ident[:UCORE, :UCORE])
                nc.vector.tensor_copy(gdecT[dch][:, :UCORE], ps[:, :UCORE])
            for dch in range(4):
                dsl = slice(dch * 128, (dch + 1) * 128)
                for i in range(3):
                    ps = psum.tile([128, 128], bf16, tag="tr", bufs=2)
                    nc.tensor.transpose(ps[:, :], genc[:, i, dsl], ident[:, :])
                    nc.vector.tensor_copy(gencT[dch][:, i * 128 : (i + 1) * 128], ps[:, :])

            # P_dec [52,640] bf16 -> SBUF->SBUF DMA relayout to row tile at
            # partitions 0/32/64
            pd_bf = persist.tile([3 * RG, V], bf16, tag="pd")
            nc.gpsimd.memset(pd_bf[:, :], 0.0)  # rows 52-53 stay zero (pad)
            ps = psum.tile([128, V], f32, tag="bc", bufs=3)
            for v0, vn in vchunks:
                for d in range(4):
                    nc.tensor.matmul(
                        ps[:UCORE, v0 : v0 + vn],
                        gdecT[d][:, :UCORE],
                        w_bf[:, 4 + d, v0 : v0 + vn],
                        start=(d == 0),
                        stop=(d == 3),
                    )
            nc.vector.tensor_copy(pd_bf[:UCORE, :], ps[:UCORE, :])
            rows = persist.tile([65, RG * V], bf16, tag="rows")
            # on sync: this dispatch waits on the pd copy inside the FIFO and
            # must not block ACT (whose FIFO feeds the pe2 copies)
            nc.sync.dma_start(rows[0:65:32, :], pd_bf[:, :])

            # P_enc (with bias): t-chunks 0,1 packed in pe2. The 44-row t-tail
            # (t 256-299) is computed TWICE: once landing on partitions 0-43
            # (pe3a) and once on partitions 64-107 (pe3b, lhsT cols 192-300
            # with rows 192-255 duplicated). The tail store alternates between
            # them per block parity so its bytes split between the even
            # (p0-63) and odd (p64-127) SDMA engine groups — otherwise the
            # even engines carry all tail descriptors and bound the loop.
            pe2 = persist.tile([128, 2, V], f32, tag="pe2")
            pe3a = persist.tile([44, V], f32, tag="pe3a")
            pe3b = persist.tile([108, V], f32, tag="pe3b")
            for i, (t0, tn) in enumerate([(0, 128), (128, 128), (256, 44), (192, 108)]):
                ps = psum.tile([128, V], f32, tag="bc", bufs=3)
                for v0, vn in vchunks:
                    for d in range(4):
                        nc.tensor.matmul(
                            ps[:tn, v0 : v0 + vn],
                            gencT[d][:, t0 : t0 + tn],
                            w_bf[:, d, v0 : v0 + vn],
                            start=(d == 0),
                            stop=False,
                        )
                    nc.tensor.matmul(
                        ps[:tn, v0 : v0 + vn],
                        ones3[0:1, :tn],
                        bias_sb[:1, v0 : v0 + vn],
                        start=False,
                        stop=True,
                    )
                if i < 2:
                    nc.scalar.copy(pe2[:, i, :], ps[:tn, :])
                elif i == 2:
                    nc.scalar.copy(pe3a[:, :], ps[:44, :])
                else:
                    nc.scalar.copy(pe3b[64:108, :], ps[64:108, :])

            # main loop: 13 blocks of 4 u
            for bi, u0 in enumerate(range(0, UCORE, UB)):
                par = bi % 2
                if par == 0:
                    bct4 = bctp.tile([44, UB, V], f32, tag="bcta", name="bct4a", bufs=2)
                    otB = outpB.tile([44, UB, V], f32, tag="otBa", name="otBa", bufs=2)
                    psl, pe3s = slice(0, 44), pe3a[:, :]
                else:
                    bct4 = bctp.tile([108, UB, V], f32, tag="bctb", name="bct4b", bufs=2)
                    otB = outpB.tile([108, UB, V], f32, tag="otBb", name="otBb", bufs=2)
                    psl, pe3s = slice(64, 108), pe3b[64:108, :]
                otA = outpA.tile([128, 2, UB, V], f32, tag="otA", name="otA")
                for j in range(UB):
                    u = u0 + j
                    g, off = u // RG, (u % RG) * V
                    ps = psum.tile([128, V], f32, tag="bc", bufs=3)
                    for c0, cn in ((0, 512), (512, V - 512)):
                        nc.tensor.matmul(
                            ps[:, c0 : c0 + cn],
                            ones3[32 * g : 32 * g + 1, :128],
                            rows[32 * g : 32 * g + 1, off + c0 : off + c0 + cn],
                            start=True,
                            stop=True,
                        )
                    nc.scalar.copy(bct4[psl, j, :], ps[psl, :])
                    nc.vector.tensor_add(
                        otA[:, :, j, :],
                        pe2[:, :, :],
                        ps[:, :].unsqueeze(1).broadcast_to([128, 2, V]),
                    )
                nc.gpsimd.tensor_add(
                    otB[psl, :, :],
                    pe3s.unsqueeze(1).broadcast_to([44, UB, V]),
                    bct4[psl, :, :],
                )
                engA, engB = (nc.sync, nc.scalar) if bi % 2 == 0 else (nc.scalar, nc.sync)
                if bi == 0:
                    # first block: store per u-pair across both rings so the
                    # first store issues right after the second DVE add
                    engA.dma_start(outA_d[bi, :, :, 0:2, :], otA[:, :, 0:2, :])
                    engB.dma_start(outA_d[bi, :, :, 2:4, :], otA[:, :, 2:4, :])
                    engA.dma_start(outB_d[bi, :, :, :], otB[psl, :, :])
                elif bi < UCORE // UB - 1:
                    engA.dma_start(outA_d[bi, :, :, :, :], otA[:, :, :, :])
                    engB.dma_start(outB_d[bi, :, :, :], otB[psl, :, :])
                else:
                    # last block: split the big store across both rings so the
                    # drain is half as long
                    engA.dma_start(outA_d[bi, :, 0, :, :], otA[:, 0, :, :])
                    engB.dma_start(outA_d[bi, :, 1, :, :], otA[:, 1, :, :])
                    engA.dma_start(outB_d[bi, :, :, :], otB[psl, :, :])

    nc.compile()
    return nc


def kernel(encoder_outputs, decoder_outputs, W_fc, b_fc):
    global LAST_RESULT
    import os

    import ml_dtypes
    from concourse.bass_utils import run_bass_kernel_spmd

    bf = ml_dtypes.bfloat16
    enc = np.asarray(encoder_outputs, dtype=np.float32)
    dec = np.asarray(decoder_outputs, dtype=np.float32)

    # enc per batch -> [128, 3, 512] with row t = c*128 + p, zero padded
    enc_pad = np.zeros((B, 384, D), dtype=np.float32)
    enc_pad[:, :T, :] = enc
    enc_tiled = np.ascontiguousarray(
        enc_pad.reshape(B, 3, 128, D).transpose(0, 2, 1, 3)
    ).astype(bf)

    # W_fc.T -> [128, 8, 640] with row d = c*128 + p
    wT = np.asarray(W_fc, dtype=np.float32).T  # (1024, 640)
    wT_tiled = np.ascontiguousarray(
        wT.reshape(8, 128, V).transpose(1, 0, 2)
    ).astype(bf)

    bias = np.asarray(b_fc, dtype=np.float32)[None, :].astype(bf)

    dec_pad = np.zeros((B, 2 * UCORE, D), dtype=np.float32)
    dec_pad[:, :U, :] = dec
    dec_pad = dec_pad.astype(bf)

    if "nc" not in _cache:
        _cache["nc"] = _build()
    nc = _cache["nc"]

    in_maps = []
    for c in range(NCORES):
        b, uh = c // 2, c % 2
        in_maps.append(
            {
                "enc": enc_tiled[b],
                "dec": np.ascontiguousarray(dec_pad[b, uh * UCORE : (uh + 1) * UCORE]),
                "wT": wT_tiled,
                "bias": bias,
            }
        )

    res = run_bass_kernel_spmd(
        nc,
        in_maps,
        list(range(NCORES)),
        trace=bool(int(os.environ.get("KJ_TRACE", "0"))),
        **RUN_KWARGS,
    )
    LAST_RESULT = res

    out = np.empty((B, T, U, V), dtype=np.float32)
    for c in range(NCORES):
        b, uh = c // 2, c % 2
        # outA (13,128,2,4,640): [bi,p,cc,j,v] -> t = cc*128+p, u = 4*bi+j
        # outB (13,44,4,640):    [bi,p,j,v]    -> t = 256+p,    u = 4*bi+j
        outA = res.results[c]["outA"]
        outB = res.results[c]["outB"]
        cut = np.empty((T, UCORE, V), dtype=np.float32)
        cut[:256] = outA.transpose(2, 1, 0, 3, 4).reshape(256, UCORE, V)
        cut[256:] = outB.transpose(1, 0, 2, 3).reshape(44, UCORE, V)
        if uh == 0:
            out[b, :, :UCORE] = cut
        else:
            out[b, :, UCORE:U] = cut[:, : U - UCORE]
    return out


# revision 42
# speedup vs baseline: 1.0323x; 1.0227x over previous
"""RNN-T JointNet kernel for 8 Trainium2 NeuronCores.

Math: out[b,t,u,:] = gelu_tanh(concat(enc[b,t], dec[b,u])) @ W_fc^T + b_fc
Since gelu is elementwise, gelu(concat(a,b)) = concat(gelu(a), gelu(b)), so
  out[b,t,u,:] = P_enc[b,t,:] + P_dec[b,u,:]
with P_enc = gelu(enc) @ W_fc[:, :512]^T + b_fc  (small matmul, (B,T,V))
     P_dec = gelu(dec) @ W_fc[:, 512:]^T         (small matmul, (B,U,V))
The dominant cost is streaming the (B,T,U,V) = 310MB f32 output to HBM
(~111us/core at 358 GB/s); everything else must hide under the stores.

Sharding: 8 cores = 4 batches x 2 u-halves. Core c -> b = c//2, u-range
[(c%2)*52, (c%2)*52+52) of U padded 101->104. Full T=300 per core.

Inputs/weights are pre-cast to bf16 and pre-tiled on the host so every
input lands in one contiguous-per-partition DMA; the weight DMA is
dispatched first since it gates the matmul phase. A burst of dummy
transposes keeps the PE HAM clock-gate open (2.4GHz) before the real
matmuls. Each P_enc/P_dec matmul group accumulates both v-chunks into one
[128,640] PSUM tile (shared tag with the loop broadcasts; 3 bufs + 2
transpose banks = 8 PSUM banks). P_dec rows are relayouted to a
partition-{0,32,64} row tile with one SBUF->SBUF DMA (no DRAM bounce).
Per u the PE broadcasts one P_dec row into PSUM (K=1 bf16 matmuls); one
DVE op adds pe2 (both 128-row t-chunks, stride-0 broadcast of the PSUM
tile) into the otA tile, ACT copies the 44-row tail slice to SBUF, and
gpsimd adds the t-tail for the whole 4-u block. Stores: one 2.62MB (4D AP
over both t-chunks) + one 0.45MB DMA per block, alternating HWDGE rings.
"""

import numpy as np

B, T, U = 4, 300, 101
D = 512
V = 640
UCORE = 52  # u rows per core (U padded to 104)
NCORES = 8
UB = 4  # u rows per store block (52 = 13 * 4)
RG = 18  # u rows per row-group partition (groups at partitions 0/32/64)
NWARM = 45  # dummy PE transposes to hold the HAM clock-gate open

LAST_RESULT = None  # BassKernelResults of the most recent run (for test.py)
RUN_KWARGS = {}  # extra kwargs test.py may inject (e.g. tmpdir for traces)

_cache = {}


def _build():
    import concourse.mybir as mybir
    from concourse import bacc, masks
    from concourse.tile import TileContext

    f32 = mybir.dt.float32
    bf16 = mybir.dt.bfloat16
    AF = mybir.ActivationFunctionType

    nc = bacc.Bacc()
    # host pre-tiled: enc[p, c, :] = gelu-input row t = c*128+p (zero-padded)
    enc_d = nc.dram_tensor("enc", [128, 3, D], bf16, kind="ExternalInput")
    dec_d = nc.dram_tensor("dec", [UCORE, D], bf16, kind="ExternalInput")
    # host pre-tiled: wT[p, c, :] = W_fc.T row d = c*128+p
    wT_d = nc.dram_tensor("wT", [128, 8, V], bf16, kind="ExternalInput")
    bias_d = nc.dram_tensor("bias", [1, V], bf16, kind="ExternalInput")
    # outputs laid out exactly like the SBUF tiles so every store is one
    # fully contiguous DRAM write (best HBM locality); host un-permutes.
    # outA[bi, p, c, j, v] = out[t = c*128 + p, u = 4*bi + j, v]
    # outB[bi, p, j, v]    = out[t = 256 + p,   u = 4*bi + j, v]
    NBLK = UCORE // UB
    outA_d = nc.dram_tensor("outA", [NBLK, 128, 2, UB, V], f32, kind="ExternalOutput")
    outB_d = nc.dram_tensor("outB", [NBLK, 44, UB, V], f32, kind="ExternalOutput")

    tchunks = [(0, 128), (128, 128), (256, 44)]
    vchunks = [(0, 512), (512, V - 512)]

    with TileContext(nc) as tc:
        with (
            tc.tile_pool(name="const", bufs=1) as constp,
            tc.tile_pool(name="work", bufs=2) as work,
            tc.tile_pool(name="persist", bufs=1) as persist,
            tc.tile_pool(name="outpA", bufs=3) as outpA,
            tc.tile_pool(name="outpB", bufs=3) as outpB,
            tc.tile_pool(name="bctp", bufs=3) as bctp,
            tc.tile_pool(name="psum", bufs=1, space="PSUM") as psum,
        ):
            # input loads: small gelu inputs first (they head the compute
            # chains), split across both HWDGE rings; w queued right behind
            dt_in = work.tile([128, D], bf16, tag="ld", name="dt_in")
            nc.sync.dma_start(dt_in[:UCORE, :], dec_d[:, :])
            et = work.tile([128, 3, D], bf16, tag="lde", name="et")
            nc.scalar.dma_start(et[:, :, :], enc_d[:, :, :])
            # w on the SWDGE (gpsimd) ring: dispatches early (no ACT table
            # loads ahead of it) and overlaps the HWDGE input loads
            w_bf = persist.tile([128, 8, V], bf16, tag="w")
            nc.gpsimd.dma_start(w_bf[:, :, :], wT_d[:, :, :])
            bias_sb = constp.tile([1, V], bf16)
            nc.scalar.dma_start(bias_sb[:], bias_d[:])

            ident = constp.tile([128, 128], bf16)
            masks.make_identity(nc, ident[:])
            # ones at base partitions 0/32/64 (matmul lhsT/rhs must share base)
            ones3 = constp.tile([65, 128], bf16)
            nc.gpsimd.memset(ones3[:], 1.0)

            # dummy PE ops: absorb the gpsimd-sem wait AND keep the PE HAM
            # activity window busy until real matmuls arrive, so they run at
            # 2.4GHz instead of the cold 1.2GHz
            warm = psum.tile([128, 128], bf16, tag="tr", bufs=2)
            for _ in range(NWARM):
                nc.tensor.transpose(warm[:, :], ident[:, :], ident[:, :])

            # gelu: dec first (heads the deeper P_dec->rows chain)
            gdec = persist.tile([128, D], bf16, tag="gdec")
            nc.scalar.activation(gdec[:UCORE, :], dt_in[:UCORE, :], AF.Gelu_apprx_tanh)
            genc = persist.tile([128, 3, D], bf16, tag="genc")
            nc.scalar.activation(genc[:, :, :], et[:, :, :], AF.Gelu_apprx_tanh)

            # transpose to [d, u] / [d, t]; psum->SBUF copies on the idle DVE
            gdecT = [persist.tile([128, UCORE], bf16, tag=f"gdecT{d}", name=f"gdecT{d}") for d in range(4)]
            gencT = [persist.tile([128, 384], bf16, tag=f"gencT{d}", name=f"gencT{d}") for d in range(4)]
            for dch in range(4):
                dsl = slice(dch * 128, (dch + 1) * 128)
                ps = psum.tile([128, 128], bf16, tag="tr", bufs=2)
                nc.tensor.transpose(ps[:, :UCORE], gdec[:UCORE, dsl], ident[:UCORE, :UCORE])
                nc.vector.tensor_copy(gdecT[dch][:, :UCORE], ps[:, :UCORE])
            for dch in range(4):
                dsl = slice(dch * 128, (dch + 1) * 128)
                for i in range(3):
                    ps = psum.tile([128, 128], bf16, tag="tr", bufs=2)
                    nc.tensor.transpose(ps[:, :], genc[:, i, dsl], ident[:, :])
                    nc.vector.tensor_copy(gencT[dch][:, i * 128 : (i + 1) * 128], ps[:, :])

            # P_dec [52,640] bf16 -> SBUF->SBUF DMA relayout to row tile at
            # partitions 0/32/64
            pd_bf = persist.tile([3 * RG, V], bf16, tag="pd")
            nc.gpsimd.memset(pd_bf[:, :], 0.0)  # rows 52-53 stay zero (pad)
            ps = psum.tile([128, V], f32, tag="bc", bufs=3)
            for v0, vn in vchunks:
                for d in range(4):
                    nc.tensor.matmul(
                        ps[:UCORE, v0 : v0 + vn],
                        gdecT[d][:, :UCORE],
                        w_bf[:, 4 + d, v0 : v0 + vn],
                        start=(d == 0),
                        stop=(d == 3),
                    )
            nc.vector.tensor_copy(pd_bf[:UCORE, :], ps[:UCORE, :])
            rows = persist.tile([65, RG * V], bf16, tag="rows")
            nc.scalar.dma_start(rows[0:65:32, :], pd_bf[:, :])

            # P_enc (with bias): t-chunks 0,1 packed in pe2. The 44-row t-tail
            # (t 256-299) is computed TWICE: once landing on partitions 0-43
            # (pe3a) and once on partitions 64-107 (pe3b, lhsT cols 192-300
            # with rows 192-255 duplicated). The tail store alternates between
            # them per block parity so its bytes split between the even
            # (p0-63) and odd (p64-127) SDMA engine groups — otherwise the
            # even engines carry all tail descriptors and bound the loop.
            pe2 = persist.tile([128, 2, V], f32, tag="pe2")
            pe3a = persist.tile([44, V], f32, tag="pe3a")
            pe3b = persist.tile([108, V], f32, tag="pe3b")
            for i, (t0, tn) in enumerate([(0, 128), (128, 128), (256, 44), (192, 108)]):
                ps = psum.tile([128, V], f32, tag="bc", bufs=3)
                for v0, vn in vchunks:
                    for d in range(4):
                        nc.tensor.matmul(
                            ps[:tn, v0 : v0 + vn],
                            gencT[d][:, t0 : t0 + tn],
                            w_bf[:, d, v0 : v0 + vn],
                            start=(d == 0),
                            stop=False,
                        )
                    nc.tensor.matmul(
                        ps[:tn, v0 : v0 + vn],
                        ones3[0:1, :tn],
                        bias_sb[:1, v0 : v0 + vn],
                        start=False,
                        stop=True,
                    )
                if i < 2:
                    nc.scalar.copy(pe2[:, i, :], ps[:tn, :])
                elif i == 2:
                    nc.scalar.copy(pe3a[:, :], ps[:44, :])
                else:
                    nc.scalar.copy(pe3b[64:108, :], ps[64:108, :])

            # main loop: 13 blocks of 4 u
            for bi, u0 in enumerate(range(0, UCORE, UB)):
                par = bi % 2
                if par == 0:
                    bct4 = bctp.tile([44, UB, V], f32, tag="bcta", name="bct4a", bufs=2)
                    otB = outpB.tile([44, UB, V], f32, tag="otBa", name="otBa", bufs=2)
                    psl, pe3s = slice(0, 44), pe3a[:, :]
                else:
                    bct4 = bctp.tile([108, UB, V], f32, tag="bctb", name="bct4b", bufs=2)
                    otB = outpB.tile([108, UB, V], f32, tag="otBb", name="otBb", bufs=2)
                    psl, pe3s = slice(64, 108), pe3b[64:108, :]
                otA = outpA.tile([128, 2, UB, V], f32, tag="otA", name="otA")
                for j in range(UB):
                    u = u0 + j
                    g, off = u // RG, (u % RG) * V
                    ps = psum.tile([128, V], f32, tag="bc", bufs=3)
                    for c0, cn in ((0, 512), (512, V - 512)):
                        nc.tensor.matmul(
                            ps[:, c0 : c0 + cn],
                            ones3[32 * g : 32 * g + 1, :128],
                            rows[32 * g : 32 * g + 1, off + c0 : off + c0 + cn],
                            start=True,
                            stop=True,
                        )
                    nc.scalar.copy(bct4[psl, j, :], ps[psl, :])
                    nc.vector.tensor_add(
                        otA[:, :, j, :],
                        pe2[:, :, :],
                        ps[:, :].unsqueeze(1).broadcast_to([128, 2, V]),
                    )
                nc.gpsimd.tensor_add(
                    otB[psl, :, :],
                    pe3s.unsqueeze(1).broadcast_to([44, UB, V]),
                    bct4[psl, :, :],
                )
                engA, engB = (nc.sync, nc.scalar) if bi % 2 == 0 else (nc.scalar, nc.sync)
                if bi == 0:
                    # first block: store per u-pair across both rings so the
                    # first store issues right after the second DVE add
                    engA.dma_start(outA_d[bi, :, :, 0:2, :], otA[:, :, 0:2, :])
                    engB.dma_start(outA_d[bi, :, :, 2:4, :], otA[:, :, 2:4, :])
                    engA.dma_start(outB_d[bi, :, :, :], otB[psl, :, :])
                elif bi < UCORE // UB - 1:
                    engA.dma_start(outA_d[bi, :, :, :, :], otA[:, :, :, :])
                    engB.dma_start(outB_d[bi, :, :, :], otB[psl, :, :])
                else:
                    # last block: split the big store across both rings so the
                    # drain is half as long
                    engA.dma_start(outA_d[bi, :, 0, :, :], otA[:, 0, :, :])
                    engB.dma_start(outA_d[bi, :, 1, :, :], otA[:, 1, :, :])
                    engA.dma_start(outB_d[bi, :, :, :], otB[psl, :, :])

    nc.compile()
    return nc


def kernel(encoder_outputs, decoder_outputs, W_fc, b_fc):
    global LAST_RESULT
    import os

    import ml_dtypes
    from concourse.bass_utils import run_bass_kernel_spmd

    bf = ml_dtypes.bfloat16
    enc = np.asarray(encoder_outputs, dtype=np.float32)
    dec = np.asarray(decoder_outputs, dtype=np.float32)

    # enc per batch -> [128, 3, 512] with row t = c*128 + p, zero padded
    enc_pad = np.zeros((B, 384, D), dtype=np.float32)
    enc_pad[:, :T, :] = enc
    enc_tiled = np.ascontiguousarray(
        enc_pad.reshape(B, 3, 128, D).transpose(0, 2, 1, 3)
    ).astype(bf)

    # W_fc.T -> [128, 8, 640] with row d = c*128 + p
    wT = np.asarray(W_fc, dtype=np.float32).T  # (1024, 640)
    wT_tiled = np.ascontiguousarray(
        wT.reshape(8, 128, V).transpose(1, 0, 2)
    ).astype(bf)

    bias = np.asarray(b_fc, dtype=np.float32)[None, :].astype(bf)

    dec_pad = np.zeros((B, 2 * UCORE, D), dtype=np.float32)
    dec_pad[:, :U, :] = dec
    dec_pad = dec_pad.astype(bf)

    if "nc" not in _cache:
        _cache["nc"] = _build()
    nc = _cache["nc"]

    in_maps = []
    for c in range(NCORES):
        b, uh = c // 2, c % 2
        in_maps.append(
            {
                "enc": enc_tiled[b],
                "dec": np.ascontiguousarray(dec_pad[b, uh * UCORE : (uh + 1) * UCORE]),
                "wT": wT_tiled,
                "bias": bias,
            }
        )

    res = run_bass_kernel_spmd(
        nc,
        in_maps,
        list(range(NCORES)),
        trace=bool(int(os.environ.get("KJ_TRACE", "0"))),
        **RUN_KWARGS,
    )
    LAST_RESULT = res

    out = np.empty((B, T, U, V), dtype=np.float32)
    for c in range(NCORES):
        b, uh = c // 2, c % 2
        # outA (13,128,2,4,640): [bi,p,cc,j,v] -> t = cc*128+p, u = 4*bi+j
        # outB (13,44,4,640):    [bi,p,j,v]    -> t = 256+p,    u = 4*bi+j
        outA = res.results[c]["outA"]
        outB = res.results[c]["outB"]
        cut = np.empty((T, UCORE, V), dtype=np.float32)
        cut[:256] = outA.transpose(2, 1, 0, 3, 4).reshape(256, UCORE, V)
        cut[256:] = outB.transpose(1, 0, 2, 3).reshape(44, UCORE, V)
        if uh == 0:
            out[b, :, :UCORE] = cut
        else:
            out[b, :, UCORE:U] = cut[:, : U - UCORE]
    return out


# revision 43
# speedup vs baseline: 1.0379x; 1.0055x over previous
"""RNN-T JointNet kernel for 8 Trainium2 NeuronCores.

Math: out[b,t,u,:] = gelu_tanh(concat(enc[b,t], dec[b,u])) @ W_fc^T + b_fc
Since gelu is elementwise, gelu(concat(a,b)) = concat(gelu(a), gelu(b)), so
  out[b,t,u,:] = P_enc[b,t,:] + P_dec[b,u,:]
with P_enc = gelu(enc) @ W_fc[:, :512]^T + b_fc  (small matmul, (B,T,V))
     P_dec = gelu(dec) @ W_fc[:, 512:]^T         (small matmul, (B,U,V))
The dominant cost is streaming the (B,T,U,V) = 310MB f32 output to HBM
(~111us/core at 358 GB/s); everything else must hide under the stores.

Sharding: 8 cores = 4 batches x 2 u-halves. Core c -> b = c//2, u-range
[(c%2)*52, (c%2)*52+52) of U padded 101->104. Full T=300 per core.

Inputs/weights are pre-cast to bf16 and pre-tiled on the host so every
input lands in one contiguous-per-partition DMA; the weight DMA goes via
the SWDGE (gpsimd) ring, which dispatches early. A burst of dummy
transposes keeps the PE HAM clock-gate open (2.4GHz) through the matmul
phase. Each P_enc/P_dec matmul group accumulates both v-chunks into one
[128,640] PSUM tile (tag shared with the loop broadcasts; 3 bufs + 2
transpose banks = 8 PSUM banks). P_dec rows are relayouted to a
partition-{0,32,64} row tile with one SBUF->SBUF DMA (no DRAM bounce).
Per u the PE broadcasts one P_dec row into PSUM (K=1 bf16 matmuls); one
DVE op adds pe2 (both 128-row t-chunks, stride-0 broadcast of the PSUM
tile) into the otA tile, ACT copies the 44-row tail slice to SBUF, and
gpsimd adds the t-tail for the whole 4-u block. Stores are fully
contiguous DRAM writes (host un-permutes): 2.62MB otA + 0.45MB otB per
block on alternating HWDGE rings. The t-tail is computed twice (partitions
0-43 and 64-107) and its store alternates between them per block parity,
splitting the tail bytes between the even/odd SDMA engine groups — with a
single placement the hot engines carried 24 10KB-descriptors per block
(9.2us) vs the 8.58us HBM floor. Measured ~147-151us/core (stores run at
~348 GB/s, 97% of the 358 GB/s per-core HBM write roofline).
"""

import numpy as np

B, T, U = 4, 300, 101
D = 512
V = 640
UCORE = 52  # u rows per core (U padded to 104)
NCORES = 8
UB = 4  # u rows per store block (52 = 13 * 4)
RG = 18  # u rows per row-group partition (groups at partitions 0/32/64)
NWARM = 45  # dummy PE transposes to hold the HAM clock-gate open

LAST_RESULT = None  # BassKernelResults of the most recent run (for test.py)
RUN_KWARGS = {}  # extra kwargs test.py may inject (e.g. tmpdir for traces)

_cache = {}


def _build():
    import concourse.mybir as mybir
    from concourse import bacc, masks
    from concourse.tile import TileContext

    f32 = mybir.dt.float32
    bf16 = mybir.dt.bfloat16
    AF = mybir.ActivationFunctionType

    nc = bacc.Bacc()
    # host pre-tiled: enc[p, c, :] = gelu-input row t = c*128+p (zero-padded)
    enc_d = nc.dram_tensor("enc", [128, 3, D], bf16, kind="ExternalInput")
    dec_d = nc.dram_tensor("dec", [UCORE, D], bf16, kind="ExternalInput")
    # host pre-tiled: wT[p, c, :] = W_fc.T row d = c*128+p
    wT_d = nc.dram_tensor("wT", [128, 8, V], bf16, kind="ExternalInput")
    bias_d = nc.dram_tensor("bias", [1, V], bf16, kind="ExternalInput")
    # outputs laid out exactly like the SBUF tiles so every store is one
    # fully contiguous DRAM write (best HBM locality); host un-permutes.
    # outA[bi, p, c, j, v] = out[t = c*128 + p, u = 4*bi + j, v]
    # outB[bi, p, j, v]    = out[t = 256 + p,   u = 4*bi + j, v]
    NBLK = UCORE // UB
    outA_d = nc.dram_tensor("outA", [NBLK, 128, 2, UB, V], f32, kind="ExternalOutput")
    outB_d = nc.dram_tensor("outB", [NBLK, 44, UB, V], f32, kind="ExternalOutput")

    tchunks = [(0, 128), (128, 128), (256, 44)]
    vchunks = [(0, 512), (512, V - 512)]

    with TileContext(nc) as tc:
        with (
            tc.tile_pool(name="const", bufs=1) as constp,
            tc.tile_pool(name="work", bufs=2) as work,
            tc.tile_pool(name="persist", bufs=1) as persist,
            tc.tile_pool(name="outpA", bufs=3) as outpA,
            tc.tile_pool(name="outpB", bufs=3) as outpB,
            tc.tile_pool(name="bctp", bufs=3) as bctp,
            tc.tile_pool(name="psum", bufs=1, space="PSUM") as psum,
        ):
            # input loads: small gelu inputs first (they head the compute
            # chains), split across both HWDGE rings; w queued right behind
            dt_in = work.tile([128, D], bf16, tag="ld", name="dt_in")
            nc.sync.dma_start(dt_in[:UCORE, :], dec_d[:, :])
            et = work.tile([128, 3, D], bf16, tag="lde", name="et")
            nc.scalar.dma_start(et[:, :, :], enc_d[:, :, :])
            # w on the SWDGE (gpsimd) ring: dispatches early (no ACT table
            # loads ahead of it) and overlaps the HWDGE input loads
            w_bf = persist.tile([128, 8, V], bf16, tag="w")
            nc.gpsimd.dma_start(w_bf[:, :, :], wT_d[:, :, :])
            bias_sb = constp.tile([1, V], bf16)
            nc.scalar.dma_start(bias_sb[:], bias_d[:])

            ident = constp.tile([128, 128], bf16)
            masks.make_identity(nc, ident[:])
            # ones at base partitions 0/32/64 (matmul lhsT/rhs must share base)
            ones3 = constp.tile([65, 128], bf16)
            nc.gpsimd.memset(ones3[:], 1.0)

            # dummy PE ops: absorb the gpsimd-sem wait AND keep the PE HAM
            # activity window busy until real matmuls arrive, so they run at
            # 2.4GHz instead of the cold 1.2GHz
            warm = psum.tile([128, 128], bf16, tag="tr", bufs=2)
            for _ in range(NWARM):
                nc.tensor.transpose(warm[:, :], ident[:, :], ident[:, :])

            # gelu: dec first (heads the deeper P_dec->rows chain)
            gdec = persist.tile([128, D], bf16, tag="gdec")
            nc.scalar.activation(gdec[:UCORE, :], dt_in[:UCORE, :], AF.Gelu_apprx_tanh)
            genc = persist.tile([128, 3, D], bf16, tag="genc")
            nc.scalar.activation(genc[:, :, :], et[:, :, :], AF.Gelu_apprx_tanh)

            # transpose to [d, u] / [d, t]; psum->SBUF copies on the idle DVE
            gdecT = [persist.tile([128, UCORE], bf16, tag=f"gdecT{d}", name=f"gdecT{d}") for d in range(4)]
            gencT = [persist.tile([128, 384], bf16, tag=f"gencT{d}", name=f"gencT{d}") for d in range(4)]
            for dch in range(4):
                dsl = slice(dch * 128, (dch + 1) * 128)
                ps = psum.tile([128, 128], bf16, tag="tr", bufs=2)
                nc.tensor.transpose(ps[:, :UCORE], gdec[:UCORE, dsl], ident[:UCORE, :UCORE])
                nc.vector.tensor_copy(gdecT[dch][:, :UCORE], ps[:, :UCORE])
            for dch in range(4):
                dsl = slice(dch * 128, (dch + 1) * 128)
                for i in range(3):
                    ps = psum.tile([128, 128], bf16, tag="tr", bufs=2)
                    nc.tensor.transpose(ps[:, :], genc[:, i, dsl], ident[:, :])
                    nc.vector.tensor_copy(gencT[dch][:, i * 128 : (i + 1) * 128], ps[:, :])

            # P_dec [52,640] bf16 -> SBUF->SBUF DMA relayout to row tile at
            # partitions 0/32/64
            pd_bf = persist.tile([3 * RG, V], bf16, tag="pd")
            nc.gpsimd.memset(pd_bf[:, :], 0.0)  # rows 52-53 stay zero (pad)
            ps = psum.tile([128, V], f32, tag="bc", bufs=3)
            for v0, vn in vchunks:
                for d in range(4):
                    nc.tensor.matmul(
                        ps[:UCORE, v0 : v0 + vn],
                        gdecT[d][:, :UCORE],
                        w_bf[:, 4 + d, v0 : v0 + vn],
                        start=(d == 0),
                        stop=(d == 3),
                    )
            nc.vector.tensor_copy(pd_bf[:UCORE, :], ps[:UCORE, :])
            rows = persist.tile([65, RG * V], bf16, tag="rows")
            nc.scalar.dma_start(rows[0:65:32, :], pd_bf[:, :])

            # P_enc (with bias): t-chunks 0,1 packed in pe2. The 44-row t-tail
            # (t 256-299) is computed TWICE: once landing on partitions 0-43
            # (pe3a) and once on partitions 64-107 (pe3b, lhsT cols 192-300
            # with rows 192-255 duplicated). The tail store alternates between
            # them per block parity so its bytes split between the even
            # (p0-63) and odd (p64-127) SDMA engine groups — otherwise the
            # even engines carry all tail descriptors and bound the loop.
            pe2 = persist.tile([128, 2, V], f32, tag="pe2")
            pe3a = persist.tile([44, V], f32, tag="pe3a")
            pe3b = persist.tile([108, V], f32, tag="pe3b")
            for i, (t0, tn) in enumerate([(0, 128), (128, 128), (256, 44), (192, 108)]):
                ps = psum.tile([128, V], f32, tag="bc", bufs=3)
                for v0, vn in vchunks:
                    for d in range(4):
                        nc.tensor.matmul(
                            ps[:tn, v0 : v0 + vn],
                            gencT[d][:, t0 : t0 + tn],
                            w_bf[:, d, v0 : v0 + vn],
                            start=(d == 0),
                            stop=False,
                        )
                    nc.tensor.matmul(
                        ps[:tn, v0 : v0 + vn],
                        ones3[0:1, :tn],
                        bias_sb[:1, v0 : v0 + vn],
                        start=False,
                        stop=True,
                    )
                if i < 2:
                    nc.scalar.copy(pe2[:, i, :], ps[:tn, :])
                elif i == 2:
                    nc.scalar.copy(pe3a[:, :], ps[:44, :])
                else:
                    nc.scalar.copy(pe3b[64:108, :], ps[64:108, :])

            # main loop: 13 blocks of 4 u
            for bi, u0 in enumerate(range(0, UCORE, UB)):
                par = bi % 2
                if par == 0:
                    bct4 = bctp.tile([44, UB, V], f32, tag="bcta", name="bct4a", bufs=2)
                    otB = outpB.tile([44, UB, V], f32, tag="otBa", name="otBa", bufs=2)
                    psl, pe3s = slice(0, 44), pe3a[:, :]
                else:
                    bct4 = bctp.tile([108, UB, V], f32, tag="bctb", name="bct4b", bufs=2)
                    otB = outpB.tile([108, UB, V], f32, tag="otBb", name="otBb", bufs=2)
                    psl, pe3s = slice(64, 108), pe3b[64:108, :]
                otA = outpA.tile([128, 2, UB, V], f32, tag="otA", name="otA")
                for j in range(UB):
                    u = u0 + j
                    g, off = u // RG, (u % RG) * V
                    ps = psum.tile([128, V], f32, tag="bc", bufs=3)
                    for c0, cn in ((0, 512), (512, V - 512)):
                        nc.tensor.matmul(
                            ps[:, c0 : c0 + cn],
                            ones3[32 * g : 32 * g + 1, :128],
                            rows[32 * g : 32 * g + 1, off + c0 : off + c0 + cn],
                            start=True,
                            stop=True,
                        )
                    nc.scalar.copy(bct4[psl, j, :], ps[psl, :])
                    nc.vector.tensor_add(
                        otA[:, :, j, :],
                        pe2[:, :, :],
                        ps[:, :].unsqueeze(1).broadcast_to([128, 2, V]),
                    )
                nc.gpsimd.tensor_add(
                    otB[psl, :, :],
                    pe3s.unsqueeze(1).broadcast_to([44, UB, V]),
                    bct4[psl, :, :],
                )
                engA, engB = (nc.sync, nc.scalar) if bi % 2 == 0 else (nc.scalar, nc.sync)
                if bi == 0:
                    # first block: store per u-pair across both rings so the
                    # first store issues right after the second DVE add
                    engA.dma_start(outA_d[bi, :, :, 0:2, :], otA[:, :, 0:2, :])
                    engB.dma_start(outA_d[bi, :, :, 2:4, :], otA[:, :, 2:4, :])
                    engA.dma_start(outB_d[bi, :, :, :], otB[psl, :, :])
                elif bi < UCORE // UB - 1:
                    engA.dma_start(outA_d[bi, :, :, :, :], otA[:, :, :, :])
                    engB.dma_start(outB_d[bi, :, :, :], otB[psl, :, :])
                else:
                    # last block: split the big store across both rings so the
                    # drain is half as long
                    engA.dma_start(outA_d[bi, :, 0, :, :], otA[:, 0, :, :])
                    engB.dma_start(outA_d[bi, :, 1, :, :], otA[:, 1, :, :])
                    engA.dma_start(outB_d[bi, :, :, :], otB[psl, :, :])

    nc.compile()
    return nc


def kernel(encoder_outputs, decoder_outputs, W_fc, b_fc):
    global LAST_RESULT
    import os

    import ml_dtypes
    from concourse.bass_utils import run_bass_kernel_spmd

    bf = ml_dtypes.bfloat16
    enc = np.asarray(encoder_outputs, dtype=np.float32)
    dec = np.asarray(decoder_outputs, dtype=np.float32)

    # enc per batch -> [128, 3, 512] with row t = c*128 + p, zero padded
    enc_pad = np.zeros((B, 384, D), dtype=np.float32)
    enc_pad[:, :T, :] = enc
    enc_tiled = np.ascontiguousarray(
        enc_pad.reshape(B, 3, 128, D).transpose(0, 2, 1, 3)
    ).astype(bf)

    # W_fc.T -> [128, 8, 640] with row d = c*128 + p
    wT = np.asarray(W_fc, dtype=np.float32).T  # (1024, 640)
    wT_tiled = np.ascontiguousarray(
        wT.reshape(8, 128, V).transpose(1, 0, 2)
    ).astype(bf)

    bias = np.asarray(b_fc, dtype=np.float32)[None, :].astype(bf)

    dec_pad = np.zeros((B, 2 * UCORE, D), dtype=np.float32)
    dec_pad[:, :U, :] = dec
    dec_pad = dec_pad.astype(bf)

    if "nc" not in _cache:
        _cache["nc"] = _build()
    nc = _cache["nc"]

    in_maps = []
    for c in range(NCORES):
        b, uh = c // 2, c % 2
        in_maps.append(
            {
                "enc": enc_tiled[b],
                "dec": np.ascontiguousarray(dec_pad[b, uh * UCORE : (uh + 1) * UCORE]),
                "wT": wT_tiled,
                "bias": bias,
            }
        )

    res = run_bass_kernel_spmd(
        nc,
        in_maps,
        list(range(NCORES)),
        trace=bool(int(os.environ.get("KJ_TRACE", "0"))),
        **RUN_KWARGS,
    )
    LAST_RESULT = res

    out = np.empty((B, T, U, V), dtype=np.float32)
    for c in range(NCORES):
        b, uh = c // 2, c % 2
        # outA (13,128,2,4,640): [bi,p,cc,j,v] -> t = cc*128+p, u = 4*bi+j
        # outB (13,44,4,640):    [bi,p,j,v]    -> t = 256+p,    u = 4*bi+j
        outA = res.results[c]["outA"]
        outB = res.results[c]["outB"]
        cut = np.empty((T, UCORE, V), dtype=np.float32)
        cut[:256] = outA.transpose(2, 1, 0, 3, 4).reshape(256, UCORE, V)
        cut[256:] = outB.transpose(1, 0, 2, 3).reshape(44, UCORE, V)
        if uh == 0:
            out[b, :, :UCORE] = cut
        else:
            out[b, :, UCORE:U] = cut[:, : U - UCORE]
    return out
